# revision 32
# baseline (speedup 1.0000x reference)
"""Trainium2 Bass kernel for batched multi-head self-attention.

Problem: x[8,1024,768], w_qkv[768,2304], b_qkv[2304] ->
         out[8,1024,768]  (12 heads, head_dim 64, scale 768**-0.5)

Sharding: data-parallel over batch; each of the 8 NeuronCores processes one
batch element end-to-end (no collectives).

Per-core pipeline, software-pipelined so the PE never waits on the Scalar
engine's exp (which otherwise rate-limits attention):
  1. Host pre-work: transpose x[b] -> xT16 [768,1024] fp16; permute w_qkv
     columns so QK features are grouped per head-pair and V features
     head-major with a ones column per head (softmax denominators fall out
     of the PV matmul).
  2. QK projection in [feature, token] orientation (fp16) -> Q^T/K^T tiles;
     V projection in [token, feature] orientation (fp16) -> [V|1] tiles.
  3. Attention runs as 12 chunks c=(pair, q-half).  Steady state issues, per
     chunk period: energy matmuls + exp for chunk c interleaved (per k-tile)
     with the PV matmuls of chunk c-1, so exp(c-1) results are ready exactly
     when PV(c-1) consumes them and the Tensor engine stays saturated (and
     the HAM clock gate stays at 2.4 GHz).  exp is written as fp16, making
     the PV moving operand full-rate.  The PV output [d+1, q] (denominator
     row included) is copied to fp16, PE-transposed back to [q, d] (fp16,
     1 cycle/row), normalized with one batched reciprocal per head, and the
     finished 128-token x 2-head block is DMAed out per chunk.

Startup: DMA descriptors issue serially (~0.7us each) on the issuing
engine, so the initial transfers are spread across Sync (x, first-needed
column halves first), Scalar (pair-0 weights; idle until the first exp)
and GpSimd (second column halves) so the first projection starts ~10us
in and never stalls afterwards.  The final period pipelines the last
chunk per-head and ships the output from both Sync and Scalar.

Measured (trace): Tensor ~85% busy at the fp16 roofline (512-row matmul
issue-to-issue = 216ns = 512/2.4GHz + NX overhead); exp on Scalar ~96us;
~167us total vs the 222us baseline.  Note: the PE clock is thermally
throttled to 2.0GHz in some runs (matmul gap 259ns instead of 216ns) —
compare kernel variants only across full-clock runs.
"""

import numpy as np

import concourse.mybir as mybir
import concourse.tile as tile
from concourse import bacc
from concourse.bass_utils import run_bass_kernel_spmd
from concourse.masks import make_identity

B, NT, D, H, HD = 8, 1024, 768, 12, 64
KC = D // 128          # 6 contraction chunks
NPAIR = H // 2         # 6 head pairs
NCH = 2 * NPAIR        # 12 chunks: (pair, q-half)
SCALE = float(D) ** -0.5
F32 = mybir.dt.float32
FP16 = mybir.dt.float16
VP_W = H * (HD + 1)    # V-plus-ones width: 12*65 = 780
HW6 = 6 * (HD + 1)     # 390: six heads of [V_h | 1]


def _build():
    nc = bacc.Bacc("TRN2", target_bir_lowering=False, debug=False, num_devices=B)

    xT16 = nc.dram_tensor("xT16", [D, NT], FP16, kind="ExternalInput")
    wqk = nc.dram_tensor("wqk", [D, 2 * D], FP16, kind="ExternalInput")
    # wv/bv are extended on the host with a zero-weight, bias-1.0 column per
    # head ([V_h | 1] layout) so the PV matmul also produces softmax
    # denominators; bqk[p, et] = bias of feature et*128+p
    wv = nc.dram_tensor("wv", [D, VP_W], FP16, kind="ExternalInput")
    bqk = nc.dram_tensor("bqk", [128, H], F32, kind="ExternalInput")
    bv = nc.dram_tensor("bv", [128, VP_W], F32, kind="ExternalInput")
    out = nc.dram_tensor("out", [NT, D], F32, kind="ExternalOutput")

    with tile.TileContext(nc) as tc:
        with (
            tc.tile_pool(name="res", bufs=1) as res,          # persistent tensors
            tc.tile_pool(name="wstream", bufs=2) as wstream,  # streamed weights
            tc.tile_pool(name="work", bufs=3) as work,
            tc.tile_pool(name="expp", bufs=16) as expp,       # 2 chunks of exp tiles
            tc.tile_pool(name="mm", bufs=2, space="PSUM") as mmp,       # 4 banks
            tc.tile_pool(name="pvpool", bufs=3, space="PSUM") as pvpool,  # 3 banks
            tc.tile_pool(name="tpp", bufs=1, space="PSUM") as tpp,        # 1 bank (fp16)
        ):
            xt16 = [res.tile([128, NT], FP16, tag=f"xt16_{k}", name=f"xt16_{k}") for k in range(KC)]
            qkt = [res.tile([128, NT], FP16, tag=f"qkt{e}", name=f"qkt{e}") for e in range(H)]
            vp = [res.tile([128, VP_W], FP16, tag=f"vp{t}", name=f"vp{t}") for t in range(8)]
            osb = [res.tile([128, D], F32, tag=f"osb{t}", name=f"osb{t}") for t in range(8)]
            bqk_sb = res.tile([128, H], F32, tag="bqk")
            bvv = res.tile([128, VP_W], F32, tag="bvv")
            ident = res.tile([128, 128], FP16, tag="ident")

            make_identity(nc, ident[:])
            # x arrives in column halves matching the first projection's
            # consumption order (tcn=0 then tcn=1), spread over three issuing
            # engines so the serial DMA-issue queues don't gate the start
            for k in range(KC):
                nc.sync.dma_start(xt16[k][:, 0:512],
                                  xT16[k * 128:(k + 1) * 128, 0:512])
            nc.gpsimd.dma_start(bqk_sb[:], bqk[:, :])
            for k in range(KC):
                nc.gpsimd.dma_start(xt16[k][:, 512:1024],
                                    xT16[k * 128:(k + 1) * 128, 512:1024])
            nc.sync.dma_start(bvv[:], bv[:, :])

            def dma_wqk(p):
                ts = [wstream.tile([128, 256], FP16, tag=f"wqk{k}",
                                   name=f"wqk{k}_{p}") for k in range(KC)]
                for k in range(KC):
                    nc.sync.dma_start(ts[k][:], wqk[k * 128:(k + 1) * 128,
                                                    p * 256:(p + 1) * 256])
                return ts

            def dma_wv(n):
                ts = [wstream.tile([128, HW6], FP16, tag=f"wv{k}",
                                   name=f"wv{k}_{n}") for k in range(KC)]
                for k in range(KC):
                    nc.sync.dma_start(ts[k][:], wv[k * 128:(k + 1) * 128,
                                                   n * HW6:(n + 1) * HW6])
                return ts

            def etile_proj(et, wt):
                # e-tile et: even = Q-pair, odd = K-pair of pair et//2; holds
                # head (et//2*2) features on partitions 0-63, head (..+1) on
                # 64-127, tokens along free dim
                i = et % 2
                ps = mmp.tile([128, NT], F32, tag="mm", name=f"psqk{et}")
                for tcn in range(2):
                    for k in range(KC):
                        nc.tensor.matmul(
                            ps[:, tcn * 512:(tcn + 1) * 512],
                            wt[k][:, i * 128:(i + 1) * 128],
                            xt16[k][:, tcn * 512:(tcn + 1) * 512],
                            start=(k == 0), stop=(k == KC - 1),
                            skip_group_check=True)
                nc.vector.tensor_scalar_add(qkt[et][:], ps[:], bqk_sb[:, et:et + 1])

            def vproj_unit(n, t, wvt):
                ps = pvpool.tile([128, 512], F32, tag="pvp", name=f"psv{n}_{t}")
                for k in range(KC):
                    nc.tensor.matmul(ps[:, 0:HW6],
                                     xt16[k][:, t * 128:(t + 1) * 128],
                                     wvt[k][:],
                                     start=(k == 0), stop=(k == KC - 1),
                                     skip_group_check=True)
                nc.vector.tensor_add(vp[t][:, n * HW6:(n + 1) * HW6],
                                     ps[:, 0:HW6], bvv[:, n * HW6:(n + 1) * HW6])

            def energy_kt(c, kt, exl):
                # energy^T[k, q] for both heads of the pair; exp via ScalarE
                # with fused *scale (no max-subtraction: |energy*scale| < ~2.5)
                p, qc = divmod(c, 2)
                eps = mmp.tile([128, NT], F32, tag="mm", name=f"eps{c}_{kt}")
                for i in range(2):
                    qrow = slice(i * HD, (i + 1) * HD)
                    nc.tensor.matmul(
                        eps[:, i * 512:(i + 1) * 512],
                        qkt[2 * p + 1][qrow, kt * 128:(kt + 1) * 128],
                        qkt[2 * p][qrow, qc * 512:(qc + 1) * 512],
                        start=True, stop=True, skip_group_check=True)
                et_sb = expp.tile([128, NT], FP16, tag="exp", name=f"ex{c}_{kt}")
                nc.scalar.activation(et_sb[:], eps[:],
                                     mybir.ActivationFunctionType.Exp,
                                     bias=0.0, scale=SCALE)
                exl.append(et_sb)

            def pv_kt(c, kt, exl, pvps):
                p, qc = divmod(c, 2)
                for i in range(2):
                    h = 2 * p + i
                    nc.tensor.matmul(
                        pvps[i][:],
                        vp[kt][:, h * (HD + 1):(h + 1) * (HD + 1)],
                        exl[kt][:, i * 512:(i + 1) * 512],
                        start=(kt == 0), stop=(kt == 7),
                        skip_group_check=True)

            def tail(c, pvps):
                # [d+1, q] -> fp16 -> PE-transpose -> normalize -> DMA out
                p, qc = divmod(c, 2)
                for i in range(2):
                    h = 2 * p + i
                    pvt = work.tile([HD + 1, 512], FP16, tag="pvt", name=f"pvt{c}_{i}")
                    nc.vector.tensor_copy(pvt[:], pvps[i][:])
                    tpt = tpp.tile([128, 512], FP16, tag="tp", name=f"tp{c}_{i}")
                    for st in range(4):
                        nc.tensor.transpose(tpt[:, st * 128:st * 128 + 65],
                                            pvt[:, st * 128:(st + 1) * 128],
                                            ident[0:HD + 1, 0:HD + 1])
                    rc = work.tile([128, 4], F32, tag="rc", name=f"rc{c}_{i}")
                    nc.vector.reciprocal(rc[:], tpt[:, HD:4 * 128:128])
                    for st in range(4):
                        tt = qc * 4 + st
                        nc.vector.tensor_scalar_mul(
                            osb[tt][:, h * HD:(h + 1) * HD],
                            tpt[:, st * 128:st * 128 + HD], rc[:, st:st + 1])
                for st in range(4):
                    tt = qc * 4 + st
                    nc.sync.dma_start(
                        out[tt * 128:(tt + 1) * 128, 2 * p * HD:(2 * p + 2) * HD],
                        osb[tt][:, 2 * p * HD:(2 * p + 2) * HD])

            # ---- preamble: weights for pair 0/1 + V weights; project pair 0
            # pair-0 weights issue from the Scalar engine (idle until the
            # first exp) to bypass the Sync engine's serial DMA-issue queue
            wqk0 = [wstream.tile([128, 256], FP16, tag=f"wqk{k}",
                                 name=f"wqk{k}_0s") for k in range(KC)]
            for k in range(KC):
                nc.scalar.dma_start(wqk0[k][:], wqk[k * 128:(k + 1) * 128, 0:256])
            wqk_t = {0: wqk0, 1: dma_wqk(1)}
            wv_t = [dma_wv(0), dma_wv(1)]
            etile_proj(0, wqk_t[0])
            etile_proj(1, wqk_t[0])

            # ---- main software pipeline over chunks ----
            # period c issues: energy+exp(c) [interleaved per kt with PV(c-1)],
            # e-tile projection c+2, and the normalize/output tail of c-1.
            # Period 0 uses the V projection (no exp dependency) as PE filler.
            ex = {}
            pvp_of = {}
            vproj_units = [(n, t) for n in range(2) for t in range(8)]
            for c in range(NCH + 1):
                p, qc = divmod(c, 2)
                if c < NCH:
                    # prefetch weights two e-tiles ahead, project one e-tile
                    et = c + 2
                    if et < H:
                        if (et % 2 == 0 and et // 2 + 1 < NPAIR
                                and (et // 2 + 1) not in wqk_t):
                            wqk_t[et // 2 + 1] = dma_wqk(et // 2 + 1)
                        etile_proj(et, wqk_t[et // 2])
                    ex[c] = []
                    if c >= 1:
                        pvp_of[c] = [
                            pvpool.tile([128, 512], F32, tag="pvp",
                                        name=f"pvp{c}_{i}")[0:HD + 1, :]
                            for i in range(2)]
                    for kt in range(8):
                        energy_kt(c, kt, ex[c])
                        if c == 0:
                            for n, t in vproj_units[2 * kt:2 * kt + 2]:
                                vproj_unit(n, t, wv_t[n])
                        else:
                            pv_kt(c - 1, kt, ex[c - 1], pvp_of[c])
                    if c >= 1:
                        tail(c - 1, pvp_of.pop(c))
                        del ex[c - 1]
                else:
                    # final period: per-head pipelining so head 0's
                    # normalize/transpose tail overlaps head 1's PV matmuls,
                    # and the last output DMAs issue from both Sync and
                    # Scalar (both idle by now) to shorten the drain
                    pc, pqc = divmod(c - 1, 2)
                    pvps = [pvpool.tile([128, 512], F32, tag="pvp",
                                        name=f"pvp{c}_{i}")[0:HD + 1, :]
                            for i in range(2)]
                    for i in range(2):
                        h = 2 * pc + i
                        for kt in range(8):
                            nc.tensor.matmul(
                                pvps[i][:],
                                vp[kt][:, h * (HD + 1):(h + 1) * (HD + 1)],
                                ex[c - 1][kt][:, i * 512:(i + 1) * 512],
                                start=(kt == 0), stop=(kt == 7),
                                skip_group_check=True)
                        o = i * 512
                        pvt = work.tile([HD + 1, 512], FP16, tag="pvt",
                                        name=f"pvtf{i}")
                        nc.vector.tensor_copy(pvt[:], pvps[i][:])
                        if i == 0:
                            ftp = tpp.tile([128, 1024], FP16, tag="tp",
                                           name="tpf")
                        tpt = ftp
                        for st in range(4):
                            nc.tensor.transpose(tpt[:, o + st * 128:o + st * 128 + 65],
                                                pvt[:, st * 128:(st + 1) * 128],
                                                ident[0:HD + 1, 0:HD + 1])
                        rc = work.tile([128, 4], F32, tag="rc", name=f"rcf{i}")
                        nc.vector.reciprocal(rc[:], tpt[:, o + HD:o + 4 * 128:128])
                        for st in range(4):
                            tt = pqc * 4 + st
                            nc.vector.tensor_scalar_mul(
                                osb[tt][:, h * HD:(h + 1) * HD],
                                tpt[:, o + st * 128:o + st * 128 + HD],
                                rc[:, st:st + 1])
                    for st in range(4):
                        tt = pqc * 4 + st
                        cols = slice(2 * pc * HD, (2 * pc + 2) * HD)
                        eng = nc.sync if st % 2 == 0 else nc.scalar
                        eng.dma_start(out[tt * 128:(tt + 1) * 128, cols],
                                      osb[tt][:, cols])

    nc.compile()
    return nc


_NC_CACHE = None


def _get_nc():
    global _NC_CACHE
    if _NC_CACHE is None:
        _NC_CACHE = _build()
    return _NC_CACHE


def _perm_indices():
    d3 = np.arange(HD) * 3
    qk_cols = []
    for p in range(NPAIR):
        for s in (0, 1):  # Q tile then K tile
            for h in (2 * p, 2 * p + 1):
                qk_cols.append(h * (HD * 3) + d3 + s)
    v_cols = [h * (HD * 3) + d3 + 2 for h in range(H)]
    return np.concatenate(qk_cols), np.concatenate(v_cols)


def make_in_maps(x, w_qkv, b_qkv):
    qk_idx, v_idx = _perm_indices()
    wqk = np.ascontiguousarray(w_qkv[:, qk_idx], dtype=np.float16)
    # [D, 780]: per head [V_h (64 cols) | zero col]; matching bias gets 1.0 in
    # the zero col so vp = x@wv + bv carries softmax-denominator ones
    wv = np.zeros((D, VP_W), dtype=np.float16)
    bv1 = np.zeros(VP_W, dtype=np.float32)
    wv_perm = np.asarray(w_qkv, dtype=np.float32)[:, v_idx]
    bv_perm = np.asarray(b_qkv, dtype=np.float32)[v_idx]
    for h in range(H):
        wv[:, h * (HD + 1):h * (HD + 1) + HD] = wv_perm[:, h * HD:(h + 1) * HD]
        bv1[h * (HD + 1):h * (HD + 1) + HD] = bv_perm[h * HD:(h + 1) * HD]
        bv1[h * (HD + 1) + HD] = 1.0
    # [128, H]: bias of QK e-tile et at partition p is bqk_perm[et*128 + p]
    bqk = np.ascontiguousarray(
        np.asarray(b_qkv, dtype=np.float32)[qk_idx].reshape(H, 128).T)
    bv = np.ascontiguousarray(np.broadcast_to(bv1, (128, VP_W)))
    return [
        {
            "xT16": np.ascontiguousarray(np.asarray(x[b], dtype=np.float16).T),
            "wqk": wqk, "wv": wv, "bqk": bqk, "bv": bv,
        }
        for b in range(B)
    ]


def kernel(x, w_qkv, b_qkv):
    nc = _get_nc()
    in_maps = make_in_maps(x, w_qkv, b_qkv)
    res = run_bass_kernel_spmd(nc, in_maps, core_ids=list(range(B)))
    return np.stack([res.results[b]["out"] for b in range(B)]).astype(np.float32)


# revision 33
# speedup vs baseline: 1.0262x; 1.0262x over previous
"""Trainium2 Bass kernel for batched multi-head self-attention.

Problem: x[8,1024,768], w_qkv[768,2304], b_qkv[2304] ->
         out[8,1024,768]  (12 heads, head_dim 64, scale 768**-0.5)

Sharding: data-parallel over batch; each of the 8 NeuronCores processes one
batch element end-to-end (no collectives).

Per-core pipeline, software-pipelined so the PE never waits on the Scalar
engine's exp (which otherwise rate-limits attention):
  1. Host pre-work: transpose x[b] -> xT16 [768,1024] fp16; permute w_qkv
     columns so QK features are grouped per head-pair and V features
     head-major with a ones column per head (softmax denominators fall out
     of the PV matmul).
  2. QK projection in [feature, token] orientation (fp16) -> Q^T/K^T tiles;
     V projection in [token, feature] orientation (fp16) -> [V|1] tiles.
  3. Attention runs as 12 chunks c=(pair, q-half).  Steady state issues, per
     chunk period: energy matmuls + exp for chunk c interleaved (per k-tile)
     with the PV matmuls of chunk c-1, so exp(c-1) results are ready exactly
     when PV(c-1) consumes them and the Tensor engine stays saturated (and
     the HAM clock gate stays at 2.4 GHz).  exp is written as fp16, making
     the PV moving operand full-rate.  The PV output [d+1, q] (denominator
     row included) is copied to fp16, PE-transposed back to [q, d] (fp16,
     1 cycle/row), normalized with one batched reciprocal per head, and the
     finished 128-token x 2-head block is DMAed out per chunk.

Startup: DMA descriptors issue serially (~0.7us each) on the issuing
engine, so the initial transfers are spread across Sync (x, first-needed
column halves first), Scalar (pair-0 weights; idle until the first exp)
and GpSimd (second column halves) so the first projection starts ~10us
in and never stalls afterwards.  The final period pipelines the last
chunk per-head and ships the output from both Sync and Scalar.

Measured (trace): Tensor ~85% busy at the fp16 roofline (512-row matmul
issue-to-issue = 216ns = 512/2.4GHz + NX overhead); exp on Scalar ~96us;
~167us total vs the 222us baseline.  Note: the PE clock is thermally
throttled to 2.0GHz in some runs (matmul gap 259ns instead of 216ns) —
compare kernel variants only across full-clock runs.
"""

import numpy as np

import concourse.mybir as mybir
import concourse.tile as tile
from concourse import bacc
from concourse.bass_utils import run_bass_kernel_spmd
from concourse.masks import make_identity

B, NT, D, H, HD = 8, 1024, 768, 12, 64
KC = D // 128          # 6 contraction chunks
NPAIR = H // 2         # 6 head pairs
NCH = 2 * NPAIR        # 12 chunks: (pair, q-half)
SCALE = float(D) ** -0.5
F32 = mybir.dt.float32
FP16 = mybir.dt.float16
VP_W = H * (HD + 1)    # V-plus-ones width: 12*65 = 780
HW6 = 6 * (HD + 1)     # 390: six heads of [V_h | 1]


def _build():
    nc = bacc.Bacc("TRN2", target_bir_lowering=False, debug=False, num_devices=B)

    xT16 = nc.dram_tensor("xT16", [D, NT], FP16, kind="ExternalInput")
    wqk = nc.dram_tensor("wqk", [D, 2 * D], FP16, kind="ExternalInput")
    # wv/bv are extended on the host with a zero-weight, bias-1.0 column per
    # head ([V_h | 1] layout) so the PV matmul also produces softmax
    # denominators; bqk[p, et] = bias of feature et*128+p
    wv = nc.dram_tensor("wv", [D, VP_W], FP16, kind="ExternalInput")
    bqk = nc.dram_tensor("bqk", [128, H], F32, kind="ExternalInput")
    bv = nc.dram_tensor("bv", [128, VP_W], F32, kind="ExternalInput")
    out = nc.dram_tensor("out", [NT, D], F32, kind="ExternalOutput")

    with tile.TileContext(nc) as tc:
        with (
            tc.tile_pool(name="res", bufs=1) as res,          # persistent tensors
            tc.tile_pool(name="wstream", bufs=2) as wstream,  # streamed weights
            tc.tile_pool(name="work", bufs=3) as work,
            tc.tile_pool(name="expp", bufs=16) as expp,       # 2 chunks of exp tiles
            tc.tile_pool(name="mm", bufs=2, space="PSUM") as mmp,       # 4 banks
            tc.tile_pool(name="pvpool", bufs=2, space="PSUM") as pvpool,  # 2 banks
            tc.tile_pool(name="tpp", bufs=2, space="PSUM") as tpp,        # 2 banks
        ):
            xt16 = [res.tile([128, NT], FP16, tag=f"xt16_{k}", name=f"xt16_{k}") for k in range(KC)]
            qkt = [res.tile([128, NT], FP16, tag=f"qkt{e}", name=f"qkt{e}") for e in range(H)]
            vp = [res.tile([128, VP_W], FP16, tag=f"vp{t}", name=f"vp{t}") for t in range(8)]
            osb = [res.tile([128, D], F32, tag=f"osb{t}", name=f"osb{t}") for t in range(8)]
            bqk_sb = res.tile([128, H], F32, tag="bqk")
            bvv = res.tile([128, VP_W], F32, tag="bvv")
            ident = res.tile([128, 128], FP16, tag="ident")

            make_identity(nc, ident[:])
            # x arrives in column halves matching the first projection's
            # consumption order (tcn=0 then tcn=1), spread over three issuing
            # engines so the serial DMA-issue queues don't gate the start
            for k in range(KC):
                nc.sync.dma_start(xt16[k][:, 0:512],
                                  xT16[k * 128:(k + 1) * 128, 0:512])
            nc.gpsimd.dma_start(bqk_sb[:], bqk[:, :])
            for k in range(KC):
                nc.gpsimd.dma_start(xt16[k][:, 512:1024],
                                    xT16[k * 128:(k + 1) * 128, 512:1024])
            nc.sync.dma_start(bvv[:], bv[:, :])

            def dma_wqk(p):
                ts = [wstream.tile([128, 256], FP16, tag=f"wqk{k}",
                                   name=f"wqk{k}_{p}") for k in range(KC)]
                for k in range(KC):
                    nc.sync.dma_start(ts[k][:], wqk[k * 128:(k + 1) * 128,
                                                    p * 256:(p + 1) * 256])
                return ts

            def dma_wv(n):
                ts = [wstream.tile([128, HW6], FP16, tag=f"wv{k}",
                                   name=f"wv{k}_{n}") for k in range(KC)]
                for k in range(KC):
                    nc.sync.dma_start(ts[k][:], wv[k * 128:(k + 1) * 128,
                                                   n * HW6:(n + 1) * HW6])
                return ts

            def etile_proj(et, wt):
                # e-tile et: even = Q-pair, odd = K-pair of pair et//2; holds
                # head (et//2*2) features on partitions 0-63, head (..+1) on
                # 64-127, tokens along free dim
                i = et % 2
                ps = mmp.tile([128, NT], F32, tag="mm", name=f"psqk{et}")
                for tcn in range(2):
                    for k in range(KC):
                        nc.tensor.matmul(
                            ps[:, tcn * 512:(tcn + 1) * 512],
                            wt[k][:, i * 128:(i + 1) * 128],
                            xt16[k][:, tcn * 512:(tcn + 1) * 512],
                            start=(k == 0), stop=(k == KC - 1),
                            skip_group_check=True)
                nc.vector.tensor_scalar_add(qkt[et][:], ps[:], bqk_sb[:, et:et + 1])

            def vproj_unit(n, t, wvt):
                ps = pvpool.tile([128, 512], F32, tag="pvp", name=f"psv{n}_{t}")
                for k in range(KC):
                    nc.tensor.matmul(ps[:, 0:HW6],
                                     xt16[k][:, t * 128:(t + 1) * 128],
                                     wvt[k][:],
                                     start=(k == 0), stop=(k == KC - 1),
                                     skip_group_check=True)
                nc.vector.tensor_add(vp[t][:, n * HW6:(n + 1) * HW6],
                                     ps[:, 0:HW6], bvv[:, n * HW6:(n + 1) * HW6])

            def energy_kt(c, kt, exl):
                # energy^T[k, q] for both heads of the pair; exp via ScalarE
                # with fused *scale (no max-subtraction: |energy*scale| < ~2.5)
                p, qc = divmod(c, 2)
                eps = mmp.tile([128, NT], F32, tag="mm", name=f"eps{c}_{kt}")
                for i in range(2):
                    qrow = slice(i * HD, (i + 1) * HD)
                    nc.tensor.matmul(
                        eps[:, i * 512:(i + 1) * 512],
                        qkt[2 * p + 1][qrow, kt * 128:(kt + 1) * 128],
                        qkt[2 * p][qrow, qc * 512:(qc + 1) * 512],
                        start=True, stop=True, skip_group_check=True)
                et_sb = expp.tile([128, NT], FP16, tag="exp", name=f"ex{c}_{kt}")
                nc.scalar.activation(et_sb[:], eps[:],
                                     mybir.ActivationFunctionType.Exp,
                                     bias=0.0, scale=SCALE)
                exl.append(et_sb)

            def pv_kt(c, kt, exl, pvps):
                p, qc = divmod(c, 2)
                for i in range(2):
                    h = 2 * p + i
                    nc.tensor.matmul(
                        pvps[i][:],
                        vp[kt][:, h * (HD + 1):(h + 1) * (HD + 1)],
                        exl[kt][:, i * 512:(i + 1) * 512],
                        start=(kt == 0), stop=(kt == 7),
                        skip_group_check=True)

            def tail(c, pvps):
                # [d+1, q] -> fp16 -> PE-transpose -> normalize -> DMA out
                p, qc = divmod(c, 2)
                for i in range(2):
                    h = 2 * p + i
                    pvt = work.tile([HD + 1, 512], FP16, tag="pvt", name=f"pvt{c}_{i}")
                    nc.vector.tensor_copy(pvt[:], pvps[i][:])
                    tpt = tpp.tile([128, 512], FP16, tag="tp", name=f"tp{c}_{i}")
                    for st in range(4):
                        nc.tensor.transpose(tpt[:, st * 128:st * 128 + 65],
                                            pvt[:, st * 128:(st + 1) * 128],
                                            ident[0:HD + 1, 0:HD + 1])
                    rc = work.tile([128, 4], F32, tag="rc", name=f"rc{c}_{i}")
                    nc.vector.reciprocal(rc[:], tpt[:, HD:4 * 128:128])
                    for st in range(4):
                        tt = qc * 4 + st
                        nc.vector.tensor_scalar_mul(
                            osb[tt][:, h * HD:(h + 1) * HD],
                            tpt[:, st * 128:st * 128 + HD], rc[:, st:st + 1])
                for st in range(4):
                    tt = qc * 4 + st
                    nc.sync.dma_start(
                        out[tt * 128:(tt + 1) * 128, 2 * p * HD:(2 * p + 2) * HD],
                        osb[tt][:, 2 * p * HD:(2 * p + 2) * HD])

            # ---- preamble: weights for pair 0/1 + V weights; project pair 0
            # pair-0 weights issue from the Scalar engine (idle until the
            # first exp) to bypass the Sync engine's serial DMA-issue queue
            wqk0 = [wstream.tile([128, 256], FP16, tag=f"wqk{k}",
                                 name=f"wqk{k}_0s") for k in range(KC)]
            for k in range(KC):
                nc.scalar.dma_start(wqk0[k][:], wqk[k * 128:(k + 1) * 128, 0:256])
            wqk_t = {0: wqk0, 1: dma_wqk(1)}
            wv_t = [dma_wv(0), dma_wv(1)]
            etile_proj(0, wqk_t[0])
            etile_proj(1, wqk_t[0])

            # ---- main software pipeline over chunks ----
            # period c issues: energy+exp(c) [interleaved per kt with PV(c-1)],
            # e-tile projection c+2, and the normalize/output tail of c-1.
            # Period 0 uses the V projection (no exp dependency) as PE filler.
            ex = {}
            pvp_of = {}
            vproj_units = [(n, t) for n in range(2) for t in range(8)]
            for c in range(NCH + 1):
                p, qc = divmod(c, 2)
                if c < NCH:
                    # prefetch weights two e-tiles ahead, project one e-tile
                    et = c + 2
                    if et < H:
                        if (et % 2 == 0 and et // 2 + 1 < NPAIR
                                and (et // 2 + 1) not in wqk_t):
                            wqk_t[et // 2 + 1] = dma_wqk(et // 2 + 1)
                        etile_proj(et, wqk_t[et // 2])
                    ex[c] = []
                    if c >= 1:
                        pvp_of[c] = [
                            pvpool.tile([128, 512], F32, tag="pvp",
                                        name=f"pvp{c}_{i}")[0:HD + 1, :]
                            for i in range(2)]
                    for kt in range(8):
                        energy_kt(c, kt, ex[c])
                        if c == 0:
                            for n, t in vproj_units[2 * kt:2 * kt + 2]:
                                vproj_unit(n, t, wv_t[n])
                        else:
                            pv_kt(c - 1, kt, ex[c - 1], pvp_of[c])
                    if c >= 1:
                        tail(c - 1, pvp_of.pop(c))
                        del ex[c - 1]
                else:
                    # final period: per-head pipelining so head 0's
                    # normalize/transpose tail overlaps head 1's PV matmuls,
                    # and the last output DMAs issue from both Sync and
                    # Scalar (both idle by now) to shorten the drain
                    pc, pqc = divmod(c - 1, 2)
                    pvps = [pvpool.tile([128, 512], F32, tag="pvp",
                                        name=f"pvp{c}_{i}")[0:HD + 1, :]
                            for i in range(2)]
                    for i in range(2):
                        h = 2 * pc + i
                        for kt in range(8):
                            nc.tensor.matmul(
                                pvps[i][:],
                                vp[kt][:, h * (HD + 1):(h + 1) * (HD + 1)],
                                ex[c - 1][kt][:, i * 512:(i + 1) * 512],
                                start=(kt == 0), stop=(kt == 7),
                                skip_group_check=True)
                        pvt = work.tile([HD + 1, 512], FP16, tag="pvt",
                                        name=f"pvtf{i}")
                        nc.vector.tensor_copy(pvt[:], pvps[i][:])
                        tpt = tpp.tile([128, 512], FP16, tag="tp",
                                       name=f"tpf{i}")
                        for st in range(4):
                            nc.tensor.transpose(tpt[:, st * 128:st * 128 + 65],
                                                pvt[:, st * 128:(st + 1) * 128],
                                                ident[0:HD + 1, 0:HD + 1])
                        rc = work.tile([128, 4], F32, tag="rc", name=f"rcf{i}")
                        nc.vector.reciprocal(rc[:], tpt[:, HD:4 * 128:128])
                        for st in range(4):
                            tt = pqc * 4 + st
                            nc.vector.tensor_scalar_mul(
                                osb[tt][:, h * HD:(h + 1) * HD],
                                tpt[:, st * 128:st * 128 + HD],
                                rc[:, st:st + 1])
                    for st in range(4):
                        tt = pqc * 4 + st
                        cols = slice(2 * pc * HD, (2 * pc + 2) * HD)
                        eng = nc.sync if st % 2 == 0 else nc.scalar
                        eng.dma_start(out[tt * 128:(tt + 1) * 128, cols],
                                      osb[tt][:, cols])

    nc.compile()
    return nc


_NC_CACHE = None


def _get_nc():
    global _NC_CACHE
    if _NC_CACHE is None:
        _NC_CACHE = _build()
    return _NC_CACHE


def _perm_indices():
    d3 = np.arange(HD) * 3
    qk_cols = []
    for p in range(NPAIR):
        for s in (0, 1):  # Q tile then K tile
            for h in (2 * p, 2 * p + 1):
                qk_cols.append(h * (HD * 3) + d3 + s)
    v_cols = [h * (HD * 3) + d3 + 2 for h in range(H)]
    return np.concatenate(qk_cols), np.concatenate(v_cols)


def make_in_maps(x, w_qkv, b_qkv):
    qk_idx, v_idx = _perm_indices()
    wqk = np.ascontiguousarray(w_qkv[:, qk_idx], dtype=np.float16)
    # [D, 780]: per head [V_h (64 cols) | zero col]; matching bias gets 1.0 in
    # the zero col so vp = x@wv + bv carries softmax-denominator ones
    wv = np.zeros((D, VP_W), dtype=np.float16)
    bv1 = np.zeros(VP_W, dtype=np.float32)
    wv_perm = np.asarray(w_qkv, dtype=np.float32)[:, v_idx]
    bv_perm = np.asarray(b_qkv, dtype=np.float32)[v_idx]
    for h in range(H):
        wv[:, h * (HD + 1):h * (HD + 1) + HD] = wv_perm[:, h * HD:(h + 1) * HD]
        bv1[h * (HD + 1):h * (HD + 1) + HD] = bv_perm[h * HD:(h + 1) * HD]
        bv1[h * (HD + 1) + HD] = 1.0
    # [128, H]: bias of QK e-tile et at partition p is bqk_perm[et*128 + p]
    bqk = np.ascontiguousarray(
        np.asarray(b_qkv, dtype=np.float32)[qk_idx].reshape(H, 128).T)
    bv = np.ascontiguousarray(np.broadcast_to(bv1, (128, VP_W)))
    return [
        {
            "xT16": np.ascontiguousarray(np.asarray(x[b], dtype=np.float16).T),
            "wqk": wqk, "wv": wv, "bqk": bqk, "bv": bv,
        }
        for b in range(B)
    ]


def kernel(x, w_qkv, b_qkv):
    nc = _get_nc()
    in_maps = make_in_maps(x, w_qkv, b_qkv)
    res = run_bass_kernel_spmd(nc, in_maps, core_ids=list(range(B)))
    return np.stack([res.results[b]["out"] for b in range(B)]).astype(np.float32)


# revision 34
# speedup vs baseline: 1.0288x; 1.0025x over previous
"""Trainium2 Bass kernel for batched multi-head self-attention.

Problem: x[8,1024,768], w_qkv[768,2304], b_qkv[2304] ->
         out[8,1024,768]  (12 heads, head_dim 64, scale 768**-0.5)

Sharding: data-parallel over batch; each of the 8 NeuronCores processes one
batch element end-to-end (no collectives).

Per-core pipeline, software-pipelined so the PE never waits on the Scalar
engine's exp (which otherwise rate-limits attention):
  1. Host pre-work: transpose x[b] -> xT16 [768,1024] fp16; permute w_qkv
     columns so QK features are grouped per head-pair and V features
     head-major with a ones column per head (softmax denominators fall out
     of the PV matmul).
  2. QK projection in [feature, token] orientation (fp16) -> Q^T/K^T tiles;
     V projection in [token, feature] orientation (fp16) -> [V|1] tiles.
  3. Attention runs as 12 chunks c=(pair, q-half).  Steady state issues, per
     chunk period: energy matmuls + exp for chunk c interleaved (per k-tile)
     with the PV matmuls of chunk c-1, so exp(c-1) results are ready exactly
     when PV(c-1) consumes them and the Tensor engine stays saturated (and
     the HAM clock gate stays at 2.4 GHz).  exp is written as fp16, making
     the PV moving operand full-rate.  The PV output [d+1, q] (denominator
     row included) is copied to fp16, PE-transposed back to [q, d] (fp16,
     1 cycle/row), normalized with one batched reciprocal per head, and the
     finished 128-token x 2-head block is DMAed out per chunk.

Startup: DMA descriptors issue serially (~0.7us each) on the issuing
engine, so the initial transfers are spread across Sync (x, first-needed
column halves first), Scalar (pair-0 weights; idle until the first exp)
and GpSimd (second column halves) so the first projection starts ~10us
in and never stalls afterwards.  The final period pipelines the last
chunk per-head and ships the output from both Sync and Scalar.

Measured (trace): Tensor ~86% busy at the fp16 roofline (512-row matmul
issue-to-issue = 216ns = 512/2.4GHz + NX overhead; the two per-head
energy matmuls dispatch concurrently in separate 64-row groups); exp on
Scalar ~96us; ~165-167us total vs the 222us baseline.  Note: the PE
clock is thermally throttled to 2.0GHz in some runs (matmul gap 259ns
instead of 216ns) — compare kernel variants only across full-clock runs
(see check_clock.py).
"""

import numpy as np

import concourse.mybir as mybir
import concourse.tile as tile
from concourse import bacc
from concourse.bass_utils import run_bass_kernel_spmd
from concourse.masks import make_identity

B, NT, D, H, HD = 8, 1024, 768, 12, 64
KC = D // 128          # 6 contraction chunks
NPAIR = H // 2         # 6 head pairs
NCH = 2 * NPAIR        # 12 chunks: (pair, q-half)
SCALE = float(D) ** -0.5
F32 = mybir.dt.float32
FP16 = mybir.dt.float16
VP_W = H * (HD + 1)    # V-plus-ones width: 12*65 = 780
HW6 = 6 * (HD + 1)     # 390: six heads of [V_h | 1]


def _build():
    nc = bacc.Bacc("TRN2", target_bir_lowering=False, debug=False, num_devices=B)

    xT16 = nc.dram_tensor("xT16", [D, NT], FP16, kind="ExternalInput")
    wqk = nc.dram_tensor("wqk", [D, 2 * D], FP16, kind="ExternalInput")
    # wv/bv are extended on the host with a zero-weight, bias-1.0 column per
    # head ([V_h | 1] layout) so the PV matmul also produces softmax
    # denominators; bqk[p, et] = bias of feature et*128+p
    wv = nc.dram_tensor("wv", [D, VP_W], FP16, kind="ExternalInput")
    bqk = nc.dram_tensor("bqk", [128, H], F32, kind="ExternalInput")
    bv = nc.dram_tensor("bv", [128, VP_W], F32, kind="ExternalInput")
    out = nc.dram_tensor("out", [NT, D], F32, kind="ExternalOutput")

    with tile.TileContext(nc) as tc:
        with (
            tc.tile_pool(name="res", bufs=1) as res,          # persistent tensors
            tc.tile_pool(name="wstream", bufs=2) as wstream,  # streamed weights
            tc.tile_pool(name="work", bufs=3) as work,
            tc.tile_pool(name="expp", bufs=16) as expp,       # 2 chunks of exp tiles
            tc.tile_pool(name="mm", bufs=2, space="PSUM") as mmp,       # 4 banks
            tc.tile_pool(name="pvpool", bufs=2, space="PSUM") as pvpool,  # 2 banks
            tc.tile_pool(name="tpp", bufs=2, space="PSUM") as tpp,        # 2 banks
        ):
            xt16 = [res.tile([128, NT], FP16, tag=f"xt16_{k}", name=f"xt16_{k}") for k in range(KC)]
            qkt = [res.tile([128, NT], FP16, tag=f"qkt{e}", name=f"qkt{e}") for e in range(H)]
            vp = [res.tile([128, VP_W], FP16, tag=f"vp{t}", name=f"vp{t}") for t in range(8)]
            osb = [res.tile([128, D], F32, tag=f"osb{t}", name=f"osb{t}") for t in range(8)]
            bqk_sb = res.tile([128, H], F32, tag="bqk")
            bvv = res.tile([128, VP_W], F32, tag="bvv")
            ident = res.tile([128, 128], FP16, tag="ident")

            make_identity(nc, ident[:])
            # x arrives in column halves matching the first projection's
            # consumption order (tcn=0 then tcn=1), spread over three issuing
            # engines so the serial DMA-issue queues don't gate the start
            for k in range(KC):
                nc.sync.dma_start(xt16[k][:, 0:512],
                                  xT16[k * 128:(k + 1) * 128, 0:512])
            nc.gpsimd.dma_start(bqk_sb[:], bqk[:, :])
            for k in range(KC):
                nc.gpsimd.dma_start(xt16[k][:, 512:1024],
                                    xT16[k * 128:(k + 1) * 128, 512:1024])
            nc.sync.dma_start(bvv[:], bv[:, :])

            def dma_wqk(p):
                ts = [wstream.tile([128, 256], FP16, tag=f"wqk{k}",
                                   name=f"wqk{k}_{p}") for k in range(KC)]
                for k in range(KC):
                    nc.sync.dma_start(ts[k][:], wqk[k * 128:(k + 1) * 128,
                                                    p * 256:(p + 1) * 256])
                return ts

            def dma_wv(n):
                ts = [wstream.tile([128, HW6], FP16, tag=f"wv{k}",
                                   name=f"wv{k}_{n}") for k in range(KC)]
                for k in range(KC):
                    nc.sync.dma_start(ts[k][:], wv[k * 128:(k + 1) * 128,
                                                   n * HW6:(n + 1) * HW6])
                return ts

            def etile_proj(et, wt):
                # e-tile et: even = Q-pair, odd = K-pair of pair et//2; holds
                # head (et//2*2) features on partitions 0-63, head (..+1) on
                # 64-127, tokens along free dim
                i = et % 2
                ps = mmp.tile([128, NT], F32, tag="mm", name=f"psqk{et}")
                for tcn in range(2):
                    for k in range(KC):
                        nc.tensor.matmul(
                            ps[:, tcn * 512:(tcn + 1) * 512],
                            wt[k][:, i * 128:(i + 1) * 128],
                            xt16[k][:, tcn * 512:(tcn + 1) * 512],
                            start=(k == 0), stop=(k == KC - 1),
                            skip_group_check=True)
                nc.vector.tensor_scalar_add(qkt[et][:], ps[:], bqk_sb[:, et:et + 1])

            def vproj_unit(n, t, wvt):
                ps = pvpool.tile([128, 512], F32, tag="pvp", name=f"psv{n}_{t}")
                for k in range(KC):
                    nc.tensor.matmul(ps[:, 0:HW6],
                                     xt16[k][:, t * 128:(t + 1) * 128],
                                     wvt[k][:],
                                     start=(k == 0), stop=(k == KC - 1),
                                     skip_group_check=True)
                nc.vector.tensor_add(vp[t][:, n * HW6:(n + 1) * HW6],
                                     ps[:, 0:HW6], bvv[:, n * HW6:(n + 1) * HW6])

            def energy_kt(c, kt, exl):
                # energy^T[k, q] for both heads of the pair; exp via ScalarE
                # with fused *scale (no max-subtraction: |energy*scale| < ~2.5)
                p, qc = divmod(c, 2)
                eps = mmp.tile([128, NT], F32, tag="mm", name=f"eps{c}_{kt}")
                for i in range(2):
                    qrow = slice(i * HD, (i + 1) * HD)
                    nc.tensor.matmul(
                        eps[:, i * 512:(i + 1) * 512],
                        qkt[2 * p + 1][qrow, kt * 128:(kt + 1) * 128],
                        qkt[2 * p][qrow, qc * 512:(qc + 1) * 512],
                        start=True, stop=True, skip_group_check=True)
                et_sb = expp.tile([128, NT], FP16, tag="exp", name=f"ex{c}_{kt}")
                nc.scalar.activation(et_sb[:], eps[:],
                                     mybir.ActivationFunctionType.Exp,
                                     bias=0.0, scale=SCALE)
                exl.append(et_sb)

            def pv_kt(c, kt, exl, pvps):
                p, qc = divmod(c, 2)
                for i in range(2):
                    h = 2 * p + i
                    nc.tensor.matmul(
                        pvps[i][:],
                        vp[kt][:, h * (HD + 1):(h + 1) * (HD + 1)],
                        exl[kt][:, i * 512:(i + 1) * 512],
                        start=(kt == 0), stop=(kt == 7),
                        skip_group_check=True)

            def tail(c, pvps):
                # [d+1, q] -> fp16 -> PE-transpose -> normalize -> DMA out
                p, qc = divmod(c, 2)
                for i in range(2):
                    h = 2 * p + i
                    pvt = work.tile([HD + 1, 512], FP16, tag="pvt", name=f"pvt{c}_{i}")
                    nc.vector.tensor_copy(pvt[:], pvps[i][:])
                    tpt = tpp.tile([128, 512], FP16, tag="tp", name=f"tp{c}_{i}")
                    for st in range(4):
                        nc.tensor.transpose(tpt[:, st * 128:st * 128 + 65],
                                            pvt[:, st * 128:(st + 1) * 128],
                                            ident[0:HD + 1, 0:HD + 1])
                    rc = work.tile([128, 4], F32, tag="rc", name=f"rc{c}_{i}")
                    nc.vector.reciprocal(rc[:], tpt[:, HD:4 * 128:128])
                    for st in range(4):
                        tt = qc * 4 + st
                        nc.vector.tensor_scalar_mul(
                            osb[tt][:, h * HD:(h + 1) * HD],
                            tpt[:, st * 128:st * 128 + HD], rc[:, st:st + 1])
                for st in range(4):
                    tt = qc * 4 + st
                    nc.sync.dma_start(
                        out[tt * 128:(tt + 1) * 128, 2 * p * HD:(2 * p + 2) * HD],
                        osb[tt][:, 2 * p * HD:(2 * p + 2) * HD])

            # ---- preamble: weights for pair 0/1 + V weights; project pair 0
            # pair-0 weights issue from the Scalar engine (idle until the
            # first exp) to bypass the Sync engine's serial DMA-issue queue
            wqk0 = [wstream.tile([128, 256], FP16, tag=f"wqk{k}",
                                 name=f"wqk{k}_0s") for k in range(KC)]
            for k in range(KC):
                nc.scalar.dma_start(wqk0[k][:], wqk[k * 128:(k + 1) * 128, 0:256])
            wqk_t = {0: wqk0, 1: dma_wqk(1)}
            wv_t = [dma_wv(0), dma_wv(1)]
            etile_proj(0, wqk_t[0])
            etile_proj(1, wqk_t[0])

            # ---- main software pipeline over chunks ----
            # period c issues: energy+exp(c) [interleaved per kt with PV(c-1)],
            # e-tile projection c+2, and the normalize/output tail of c-1.
            # Period 0 uses the V projection (no exp dependency) as PE filler.
            ex = {}
            pvp_of = {}
            vproj_units = [(n, t) for n in range(2) for t in range(8)]
            for c in range(NCH + 1):
                p, qc = divmod(c, 2)
                if c < NCH:
                    # prefetch weights two e-tiles ahead, project one e-tile
                    et = c + 2
                    if et < H:
                        if (et % 2 == 0 and et // 2 + 1 < NPAIR
                                and (et // 2 + 1) not in wqk_t):
                            wqk_t[et // 2 + 1] = dma_wqk(et // 2 + 1)
                        etile_proj(et, wqk_t[et // 2])
                    ex[c] = []
                    if c >= 1:
                        pvp_of[c] = [
                            pvpool.tile([128, 512], F32, tag="pvp",
                                        name=f"pvp{c}_{i}")[0:HD + 1, :]
                            for i in range(2)]
                    for kt in range(8):
                        energy_kt(c, kt, ex[c])
                        if c == 0:
                            for n, t in vproj_units[2 * kt:2 * kt + 2]:
                                vproj_unit(n, t, wv_t[n])
                        else:
                            pv_kt(c - 1, kt, ex[c - 1], pvp_of[c])
                    if c >= 1:
                        tail(c - 1, pvp_of.pop(c))
                        del ex[c - 1]
                else:
                    # final period: per-head pipelining so head 0's
                    # normalize/transpose tail overlaps head 1's PV matmuls,
                    # and the last output DMAs issue from both Sync and
                    # Scalar (both idle by now) to shorten the drain
                    pc, pqc = divmod(c - 1, 2)
                    pvps = [pvpool.tile([128, 512], F32, tag="pvp",
                                        name=f"pvp{c}_{i}")[0:HD + 1, :]
                            for i in range(2)]
                    for i in range(2):
                        h = 2 * pc + i
                        for kt in range(8):
                            nc.tensor.matmul(
                                pvps[i][:],
                                vp[kt][:, h * (HD + 1):(h + 1) * (HD + 1)],
                                ex[c - 1][kt][:, i * 512:(i + 1) * 512],
                                start=(kt == 0), stop=(kt == 7),
                                skip_group_check=True)
                        pvt = work.tile([HD + 1, 512], FP16, tag="pvt",
                                        name=f"pvtf{i}")
                        nc.vector.tensor_copy(pvt[:], pvps[i][:])
                        tpt = tpp.tile([128, 512], FP16, tag="tp",
                                       name=f"tpf{i}")
                        for st in range(4):
                            nc.tensor.transpose(tpt[:, st * 128:st * 128 + 65],
                                                pvt[:, st * 128:(st + 1) * 128],
                                                ident[0:HD + 1, 0:HD + 1])
                        rc = work.tile([128, 4], F32, tag="rc", name=f"rcf{i}")
                        nc.vector.reciprocal(rc[:], tpt[:, HD:4 * 128:128])
                        for st in range(4):
                            tt = pqc * 4 + st
                            nc.vector.tensor_scalar_mul(
                                osb[tt][:, h * HD:(h + 1) * HD],
                                tpt[:, st * 128:st * 128 + HD],
                                rc[:, st:st + 1])
                    for st in range(4):
                        tt = pqc * 4 + st
                        cols = slice(2 * pc * HD, (2 * pc + 2) * HD)
                        eng = nc.sync if st % 2 == 0 else nc.scalar
                        eng.dma_start(out[tt * 128:(tt + 1) * 128, cols],
                                      osb[tt][:, cols])

    nc.compile()
    return nc


_NC_CACHE = None


def _get_nc():
    global _NC_CACHE
    if _NC_CACHE is None:
        _NC_CACHE = _build()
    return _NC_CACHE


def _perm_indices():
    d3 = np.arange(HD) * 3
    qk_cols = []
    for p in range(NPAIR):
        for s in (0, 1):  # Q tile then K tile
            for h in (2 * p, 2 * p + 1):
                qk_cols.append(h * (HD * 3) + d3 + s)
    v_cols = [h * (HD * 3) + d3 + 2 for h in range(H)]
    return np.concatenate(qk_cols), np.concatenate(v_cols)


def make_in_maps(x, w_qkv, b_qkv):
    qk_idx, v_idx = _perm_indices()
    wqk = np.ascontiguousarray(w_qkv[:, qk_idx], dtype=np.float16)
    # [D, 780]: per head [V_h (64 cols) | zero col]; matching bias gets 1.0 in
    # the zero col so vp = x@wv + bv carries softmax-denominator ones
    wv = np.zeros((D, VP_W), dtype=np.float16)
    bv1 = np.zeros(VP_W, dtype=np.float32)
    wv_perm = np.asarray(w_qkv, dtype=np.float32)[:, v_idx]
    bv_perm = np.asarray(b_qkv, dtype=np.float32)[v_idx]
    for h in range(H):
        wv[:, h * (HD + 1):h * (HD + 1) + HD] = wv_perm[:, h * HD:(h + 1) * HD]
        bv1[h * (HD + 1):h * (HD + 1) + HD] = bv_perm[h * HD:(h + 1) * HD]
        bv1[h * (HD + 1) + HD] = 1.0
    # [128, H]: bias of QK e-tile et at partition p is bqk_perm[et*128 + p]
    bqk = np.ascontiguousarray(
        np.asarray(b_qkv, dtype=np.float32)[qk_idx].reshape(H, 128).T)
    bv = np.ascontiguousarray(np.broadcast_to(bv1, (128, VP_W)))
    return [
        {
            "xT16": np.ascontiguousarray(np.asarray(x[b], dtype=np.float16).T),
            "wqk": wqk, "wv": wv, "bqk": bqk, "bv": bv,
        }
        for b in range(B)
    ]


def kernel(x, w_qkv, b_qkv):
    nc = _get_nc()
    in_maps = make_in_maps(x, w_qkv, b_qkv)
    res = run_bass_kernel_spmd(nc, in_maps, core_ids=list(range(B)))
    return np.stack([res.results[b]["out"] for b in range(B)]).astype(np.float32)


# revision 35
# speedup vs baseline: 1.0460x; 1.0167x over previous
"""Trainium2 Bass kernel for batched multi-head self-attention.

Problem: x[8,1024,768], w_qkv[768,2304], b_qkv[2304] ->
         out[8,1024,768]  (12 heads, head_dim 64, scale 768**-0.5)

Sharding: data-parallel over batch; each of the 8 NeuronCores processes one
batch element end-to-end (no collectives).

Per-core pipeline, software-pipelined so the PE never waits on the Scalar
engine's exp (which otherwise rate-limits attention):
  1. Host pre-work: transpose x[b] -> xT16 [768,1024] fp16; permute w_qkv
     columns so QK features are grouped per head-pair and V features
     head-major with a ones column per head (softmax denominators fall out
     of the PV matmul).
  2. QK projection in [feature, token] orientation (fp16) -> Q^T/K^T tiles;
     V projection in [token, feature] orientation (fp16) -> [V|1] tiles.
  3. Attention runs as 12 chunks c=(pair, q-half).  Steady state issues, per
     chunk period: energy matmuls + exp for chunk c interleaved (per k-tile)
     with the PV matmuls of chunk c-1, so exp(c-1) results are ready exactly
     when PV(c-1) consumes them and the Tensor engine stays saturated (and
     the HAM clock gate stays at 2.4 GHz).  exp is written as fp16, making
     the PV moving operand full-rate.  The PV output [d+1, q] (denominator
     row included) is copied to fp16, PE-transposed back to [q, d] (fp16,
     1 cycle/row), normalized with one batched reciprocal per head, and the
     finished 128-token x 2-head block is DMAed out per chunk.

Startup: DMA descriptors issue serially (~0.7us each) on the issuing
engine, so the initial transfers are spread across Sync (x, first-needed
column halves first), Scalar (pair-0 weights; idle until the first exp)
and GpSimd (second column halves) so the first projection starts ~10us
in and never stalls afterwards.  The final period pipelines the last
chunk per-head and ships the output from both Sync and Scalar.

Measured (trace): Tensor ~85% busy at the fp16 roofline (512-row matmul
issue-to-issue = 216ns = 512/2.4GHz + NX overhead); exp on Scalar ~96us;
~167us total vs the 222us baseline.  Note: the PE clock is thermally
throttled to 2.0GHz in some runs (matmul gap 259ns instead of 216ns) —
compare kernel variants only across full-clock runs.
"""

import numpy as np

import concourse.mybir as mybir
import concourse.tile as tile
from concourse import bacc
from concourse.bass_utils import run_bass_kernel_spmd
from concourse.masks import make_identity

B, NT, D, H, HD = 8, 1024, 768, 12, 64
KC = D // 128          # 6 contraction chunks
NPAIR = H // 2         # 6 head pairs
NCH = 2 * NPAIR        # 12 chunks: (pair, q-half)
SCALE = float(D) ** -0.5
F32 = mybir.dt.float32
FP16 = mybir.dt.float16
VP_W = H * (HD + 1)    # V-plus-ones width: 12*65 = 780
HW6 = 6 * (HD + 1)     # 390: six heads of [V_h | 1]


def _build():
    nc = bacc.Bacc("TRN2", target_bir_lowering=False, debug=False, num_devices=B)

    xT16 = nc.dram_tensor("xT16", [D, NT], FP16, kind="ExternalInput")
    wqk = nc.dram_tensor("wqk", [D, 2 * D], FP16, kind="ExternalInput")
    # wv/bv are extended on the host with a zero-weight, bias-1.0 column per
    # head ([V_h | 1] layout) so the PV matmul also produces softmax
    # denominators; bqk[p, et] = bias of feature et*128+p
    wv = nc.dram_tensor("wv", [D, VP_W], FP16, kind="ExternalInput")
    bqk = nc.dram_tensor("bqk", [128, H], F32, kind="ExternalInput")
    bv = nc.dram_tensor("bv", [128, VP_W], F32, kind="ExternalInput")
    out = nc.dram_tensor("out", [NT, D], F32, kind="ExternalOutput")

    with tile.TileContext(nc) as tc:
        with (
            tc.tile_pool(name="res", bufs=1) as res,          # persistent tensors
            tc.tile_pool(name="wstream", bufs=2) as wstream,  # streamed weights
            tc.tile_pool(name="work", bufs=3) as work,
            tc.tile_pool(name="expp", bufs=16) as expp,       # 2 chunks of exp tiles
            tc.tile_pool(name="mm", bufs=2, space="PSUM") as mmp,       # 4 banks
            tc.tile_pool(name="pvpool", bufs=2, space="PSUM") as pvpool,  # 2 banks
            tc.tile_pool(name="tpp", bufs=1, space="PSUM") as tpp,        # 1 bank (fp16)
            tc.tile_pool(name="vps", bufs=1, space="PSUM") as vps,        # 1 bank
        ):
            xt16 = [res.tile([128, NT], FP16, tag=f"xt16_{k}", name=f"xt16_{k}") for k in range(KC)]
            qkt = [res.tile([128, NT], FP16, tag=f"qkt{e}", name=f"qkt{e}") for e in range(H)]
            vp = [res.tile([128, VP_W], FP16, tag=f"vp{t}", name=f"vp{t}") for t in range(8)]
            osb = [res.tile([128, D], F32, tag=f"osb{t}", name=f"osb{t}") for t in range(8)]
            bqk_sb = res.tile([128, H], F32, tag="bqk")
            bvv = res.tile([128, VP_W], F32, tag="bvv")
            ident = res.tile([128, 128], FP16, tag="ident")

            make_identity(nc, ident[:])
            # x arrives in column halves matching the first projection's
            # consumption order (tcn=0 then tcn=1), spread over three issuing
            # engines so the serial DMA-issue queues don't gate the start
            for k in range(KC):
                nc.sync.dma_start(xt16[k][:, 0:512],
                                  xT16[k * 128:(k + 1) * 128, 0:512])
            nc.gpsimd.dma_start(bqk_sb[:], bqk[:, :])
            for k in range(KC):
                nc.gpsimd.dma_start(xt16[k][:, 512:1024],
                                    xT16[k * 128:(k + 1) * 128, 512:1024])
            nc.sync.dma_start(bvv[:], bv[:, :])

            def dma_wqk(p):
                ts = [wstream.tile([128, 256], FP16, tag=f"wqk{k}",
                                   name=f"wqk{k}_{p}") for k in range(KC)]
                for k in range(KC):
                    nc.sync.dma_start(ts[k][:], wqk[k * 128:(k + 1) * 128,
                                                    p * 256:(p + 1) * 256])
                return ts

            def dma_wv(n):
                ts = [wstream.tile([128, HW6], FP16, tag=f"wv{k}",
                                   name=f"wv{k}_{n}") for k in range(KC)]
                for k in range(KC):
                    nc.sync.dma_start(ts[k][:], wv[k * 128:(k + 1) * 128,
                                                   n * HW6:(n + 1) * HW6])
                return ts

            def etile_proj(et, wt):
                # e-tile et: even = Q-pair, odd = K-pair of pair et//2; holds
                # head (et//2*2) features on partitions 0-63, head (..+1) on
                # 64-127, tokens along free dim
                i = et % 2
                ps = mmp.tile([128, NT], F32, tag="mm", name=f"psqk{et}")
                for tcn in range(2):
                    for k in range(KC):
                        nc.tensor.matmul(
                            ps[:, tcn * 512:(tcn + 1) * 512],
                            wt[k][:, i * 128:(i + 1) * 128],
                            xt16[k][:, tcn * 512:(tcn + 1) * 512],
                            start=(k == 0), stop=(k == KC - 1),
                            skip_group_check=True)
                nc.vector.tensor_scalar_add(qkt[et][:], ps[:], bqk_sb[:, et:et + 1])

            def vproj_unit(n, t, wvt):
                ps = vps.tile([128, 512], F32, tag="vps", name=f"psv{n}_{t}")
                for k in range(KC):
                    nc.tensor.matmul(ps[:, 0:HW6],
                                     xt16[k][:, t * 128:(t + 1) * 128],
                                     wvt[k][:],
                                     start=(k == 0), stop=(k == KC - 1),
                                     skip_group_check=True)
                nc.vector.tensor_add(vp[t][:, n * HW6:(n + 1) * HW6],
                                     ps[:, 0:HW6], bvv[:, n * HW6:(n + 1) * HW6])

            def energy_kt(c, kt, exl):
                # energy^T[k, q] for both heads of the pair; exp via ScalarE
                # with fused *scale (no max-subtraction: |energy*scale| < ~2.5)
                p, qc = divmod(c, 2)
                eps = mmp.tile([128, NT], F32, tag="mm", name=f"eps{c}_{kt}")
                for i in range(2):
                    qrow = slice(i * HD, (i + 1) * HD)
                    nc.tensor.matmul(
                        eps[:, i * 512:(i + 1) * 512],
                        qkt[2 * p + 1][qrow, kt * 128:(kt + 1) * 128],
                        qkt[2 * p][qrow, qc * 512:(qc + 1) * 512],
                        start=True, stop=True, skip_group_check=True)
                et_sb = expp.tile([128, NT], FP16, tag="exp", name=f"ex{c}_{kt}")
                nc.scalar.activation(et_sb[:], eps[:],
                                     mybir.ActivationFunctionType.Exp,
                                     bias=0.0, scale=SCALE)
                exl.append(et_sb)

            def pv_kt(c, kt, exl, pvps):
                p, qc = divmod(c, 2)
                for i in range(2):
                    h = 2 * p + i
                    nc.tensor.matmul(
                        pvps[i][:],
                        vp[kt][:, h * (HD + 1):(h + 1) * (HD + 1)],
                        exl[kt][:, i * 512:(i + 1) * 512],
                        start=(kt == 0), stop=(kt == 7),
                        skip_group_check=True)

            def tail(c, pvps):
                # [d+1, q] -> fp16 -> PE-transpose -> normalize -> DMA out
                p, qc = divmod(c, 2)
                for i in range(2):
                    h = 2 * p + i
                    pvt = work.tile([HD + 1, 512], FP16, tag="pvt", name=f"pvt{c}_{i}")
                    nc.vector.tensor_copy(pvt[:], pvps[i][:])
                    tpt = tpp.tile([128, 512], FP16, tag="tp", name=f"tp{c}_{i}")
                    for st in range(4):
                        nc.tensor.transpose(tpt[:, st * 128:st * 128 + 65],
                                            pvt[:, st * 128:(st + 1) * 128],
                                            ident[0:HD + 1, 0:HD + 1])
                    rc = work.tile([128, 4], F32, tag="rc", name=f"rc{c}_{i}")
                    nc.vector.reciprocal(rc[:], tpt[:, HD:4 * 128:128])
                    for st in range(4):
                        tt = qc * 4 + st
                        nc.vector.tensor_scalar_mul(
                            osb[tt][:, h * HD:(h + 1) * HD],
                            tpt[:, st * 128:st * 128 + HD], rc[:, st:st + 1])
                for st in range(4):
                    tt = qc * 4 + st
                    nc.sync.dma_start(
                        out[tt * 128:(tt + 1) * 128, 2 * p * HD:(2 * p + 2) * HD],
                        osb[tt][:, 2 * p * HD:(2 * p + 2) * HD])

            # ---- preamble: weights for pair 0/1 + V weights; project pair 0
            # pair-0 weights issue from the Scalar engine (idle until the
            # first exp) to bypass the Sync engine's serial DMA-issue queue
            wqk0 = [wstream.tile([128, 256], FP16, tag=f"wqk{k}",
                                 name=f"wqk{k}_0s") for k in range(KC)]
            for k in range(KC):
                nc.scalar.dma_start(wqk0[k][:], wqk[k * 128:(k + 1) * 128, 0:256])
            wqk_t = {0: wqk0, 1: dma_wqk(1)}
            wv_t = [dma_wv(0), dma_wv(1)]
            etile_proj(0, wqk_t[0])
            etile_proj(1, wqk_t[0])

            # ---- main software pipeline over chunks ----
            # period c issues: energy+exp(c) [interleaved per kt with PV(c-1)],
            # e-tile projection c+2, and the normalize/output tail of c-1.
            # Period 0 uses the V projection (no exp dependency) as PE filler.
            ex = {}
            pvp_of = {}
            # V-proj group n=0 (heads 0-5, needed by PV from period 1) fills
            # period 0; group n=1 (heads 6-11, needed from period 7) spreads
            # over periods 1-5 as Tensor filler for the exp-paced kt slots
            spread = {}
            for j in range(8):
                g = j * 5
                spread[(1 + g // 8, g % 8)] = (1, j)
            for c in range(NCH + 1):
                p, qc = divmod(c, 2)
                if c < NCH:
                    # prefetch weights two e-tiles ahead, project one e-tile
                    et = c + 2
                    if et < H:
                        if (et % 2 == 0 and et // 2 + 1 < NPAIR
                                and (et // 2 + 1) not in wqk_t):
                            wqk_t[et // 2 + 1] = dma_wqk(et // 2 + 1)
                        etile_proj(et, wqk_t[et // 2])
                    ex[c] = []
                    if c >= 1:
                        pvp_of[c] = [
                            pvpool.tile([128, 512], F32, tag="pvp",
                                        name=f"pvp{c}_{i}")[0:HD + 1, :]
                            for i in range(2)]
                    for kt in range(8):
                        energy_kt(c, kt, ex[c])
                        if c == 0:
                            vproj_unit(0, kt, wv_t[0])
                        else:
                            pv_kt(c - 1, kt, ex[c - 1], pvp_of[c])
                        if (c, kt) in spread:
                            n, t = spread.pop((c, kt))
                            vproj_unit(n, t, wv_t[n])
                    if c >= 1:
                        tail(c - 1, pvp_of.pop(c))
                        del ex[c - 1]
                else:
                    # final period: per-head pipelining so head 0's
                    # normalize/transpose tail overlaps head 1's PV matmuls,
                    # and the last output DMAs issue from both Sync and
                    # Scalar (both idle by now) to shorten the drain
                    pc, pqc = divmod(c - 1, 2)
                    pvps = [pvpool.tile([128, 512], F32, tag="pvp",
                                        name=f"pvp{c}_{i}")[0:HD + 1, :]
                            for i in range(2)]
                    for i in range(2):
                        h = 2 * pc + i
                        for kt in range(8):
                            nc.tensor.matmul(
                                pvps[i][:],
                                vp[kt][:, h * (HD + 1):(h + 1) * (HD + 1)],
                                ex[c - 1][kt][:, i * 512:(i + 1) * 512],
                                start=(kt == 0), stop=(kt == 7),
                                skip_group_check=True)
                        o = i * 512
                        pvt = work.tile([HD + 1, 512], FP16, tag="pvt",
                                        name=f"pvtf{i}")
                        nc.vector.tensor_copy(pvt[:], pvps[i][:])
                        if i == 0:
                            ftp = tpp.tile([128, 1024], FP16, tag="tp",
                                           name="tpf")
                        tpt = ftp
                        for st in range(4):
                            nc.tensor.transpose(tpt[:, o + st * 128:o + st * 128 + 65],
                                                pvt[:, st * 128:(st + 1) * 128],
                                                ident[0:HD + 1, 0:HD + 1])
                        rc = work.tile([128, 4], F32, tag="rc", name=f"rcf{i}")
                        nc.vector.reciprocal(rc[:], tpt[:, o + HD:o + 4 * 128:128])
                        for st in range(4):
                            tt = pqc * 4 + st
                            nc.vector.tensor_scalar_mul(
                                osb[tt][:, h * HD:(h + 1) * HD],
                                tpt[:, o + st * 128:o + st * 128 + HD],
                                rc[:, st:st + 1])
                    for st in range(4):
                        tt = pqc * 4 + st
                        cols = slice(2 * pc * HD, (2 * pc + 2) * HD)
                        eng = nc.sync if st % 2 == 0 else nc.scalar
                        eng.dma_start(out[tt * 128:(tt + 1) * 128, cols],
                                      osb[tt][:, cols])

    nc.compile()
    return nc


_NC_CACHE = None


def _get_nc():
    global _NC_CACHE
    if _NC_CACHE is None:
        _NC_CACHE = _build()
    return _NC_CACHE


def _perm_indices():
    d3 = np.arange(HD) * 3
    qk_cols = []
    for p in range(NPAIR):
        for s in (0, 1):  # Q tile then K tile
            for h in (2 * p, 2 * p + 1):
                qk_cols.append(h * (HD * 3) + d3 + s)
    v_cols = [h * (HD * 3) + d3 + 2 for h in range(H)]
    return np.concatenate(qk_cols), np.concatenate(v_cols)


def make_in_maps(x, w_qkv, b_qkv):
    qk_idx, v_idx = _perm_indices()
    wqk = np.ascontiguousarray(w_qkv[:, qk_idx], dtype=np.float16)
    # [D, 780]: per head [V_h (64 cols) | zero col]; matching bias gets 1.0 in
    # the zero col so vp = x@wv + bv carries softmax-denominator ones
    wv = np.zeros((D, VP_W), dtype=np.float16)
    bv1 = np.zeros(VP_W, dtype=np.float32)
    wv_perm = np.asarray(w_qkv, dtype=np.float32)[:, v_idx]
    bv_perm = np.asarray(b_qkv, dtype=np.float32)[v_idx]
    for h in range(H):
        wv[:, h * (HD + 1):h * (HD + 1) + HD] = wv_perm[:, h * HD:(h + 1) * HD]
        bv1[h * (HD + 1):h * (HD + 1) + HD] = bv_perm[h * HD:(h + 1) * HD]
        bv1[h * (HD + 1) + HD] = 1.0
    # [128, H]: bias of QK e-tile et at partition p is bqk_perm[et*128 + p]
    bqk = np.ascontiguousarray(
        np.asarray(b_qkv, dtype=np.float32)[qk_idx].reshape(H, 128).T)
    bv = np.ascontiguousarray(np.broadcast_to(bv1, (128, VP_W)))
    return [
        {
            "xT16": np.ascontiguousarray(np.asarray(x[b], dtype=np.float16).T),
            "wqk": wqk, "wv": wv, "bqk": bqk, "bv": bv,
        }
        for b in range(B)
    ]


def kernel(x, w_qkv, b_qkv):
    nc = _get_nc()
    in_maps = make_in_maps(x, w_qkv, b_qkv)
    res = run_bass_kernel_spmd(nc, in_maps, core_ids=list(range(B)))
    return np.stack([res.results[b]["out"] for b in range(B)]).astype(np.float32)
